# revision 4
# baseline (speedup 1.0000x reference)
"""ChebConv GNN (3 layers, K=5) on 8 Trainium2 NeuronCores.

Strategy (hardcoded for N=100000, E=1600000, F=128/128/32/40, K=5):
  - Clenshaw evaluation of sum_k T_k(L) h W_k  -> exactly K-1=4 sparse
    propagations per layer, each on an F_out-wide operand.
  - 1D node sharding: 128-node windows assigned to (core, pass, wpos)
    slots, balanced by edge count.  Every core runs the IDENTICAL program
    (SPMD); all per-core structure lives in input tables.
  - Propagation (segment-sum over dst-sorted edges) via PE matmuls:
    psum[window] += S_tile^T @ G_tile, where G_tile = dma_gather of 128
    source rows and S_tile[e, d] = (iota[d]==dstl[e]) * w[e] built by one
    fused DVE tensor_scalar op.
  - All propagation operands in bf16.  L1 table rows are 256B (128 bf16).
    L2/L3 table rows are 128B (64 bf16), gathered with 256B elements via
    even/odd pair views (the second half of each gathered element is
    garbage from the next row and is never read by the S-matmul).
  - The gathered table is ordered (pass-half, core, slot) so each
    Clenshaw step's AllGather splits into two half-collectives: half 0
    is issued after pass 9 and overlaps passes 10-19; next-step gathers
    on chunks 0/1 depend only on half 0.
  - C_k = h @ W_k accumulated into the same PSUM tile (start=True).
  - For_i hardware loops over passes with a one-pass-shifted gather
    pipeline (gather pass i+1/i+2 while computing pass i/i+1).
"""

import sys
import os

sys.path.insert(0, "/opt/trn_rl_repo")

import numpy as np

# ---------------- problem constants (hardcoded; kernel.py must be
# self-contained and may not read spec.json/reference.py) ----------------
N = 100_000
E = 1_600_000
FIN = 128
HID = 128
F2 = 32
OUT = 40
K = 5

NCORES = 8
P = 128
GW = (N + P - 1) // P          # 782 global 128-node windows
WPP = 5                        # windows per pass
NPASS = 20                     # passes per propagation
WPC = WPP * NPASS              # 100 window slots per core
NR = WPC * P                   # 12800 rows per core shard
HROWS = NR // 2                # 6400 rows per AG half (passes 0-9 / 10-19)
TROWS = NCORES * NR            # 102400 rows in gathered table
THALF = TROWS // 2             # 51200 rows per table half
NCHUNK = 4                     # gather chunks (int16 idx < 32768)
CH = TROWS // NCHUNK           # 25600

# per-layer config
LAYER_FOUT = {1: HID, 2: F2, 3: OUT}
LAYER_FIN = {1: FIN, 2: HID, 3: F2}
F_PAD = {1: 128, 2: 64, 3: 64}     # stage/psum/tsh row width (elements)
# table row bytes: L1 256B (128 bf16), L2/3 128B (64 bf16)
ROW_B = {1: 256, 2: 128, 3: 128}
# gather elem is always 256B = 128 bf16; for L2/3 the second 64 are junk
GELEM = 128
PROP_DT = {1: os.environ.get("CHEB_L1_DT", "bf16"), 2: "bf16", 3: "bf16"}
# debug: truncate after this many Clenshaw steps (-1 = full)
TRUNC = int(os.environ.get("CHEB_TRUNC", "-1"))


# =====================================================================
# Host-side graph preprocessing
# =====================================================================
def _prep(edge_index: np.ndarray):
    src = edge_index[0].astype(np.int64)
    dst = edge_index[1].astype(np.int64)

    deg = np.bincount(src, minlength=N).astype(np.float32)
    dis = np.where(deg > 0, 1.0 / np.sqrt(np.maximum(deg, 1.0)), 0.0).astype(
        np.float32
    )
    w1 = (-dis[src] * dis[dst]).astype(np.float32)

    # ----- window -> (core, pass, wpos) assignment, balanced by size -----
    gdst = dst // P
    wcnt = np.bincount(gdst, minlength=GW)
    order = np.argsort(-wcnt, kind="stable")          # ranks: big first
    SLOT_G = NCORES * NPASS
    core_of_w = np.full(GW, -1, np.int64)
    pass_of_w = np.full(GW, -1, np.int64)
    wpos_of_w = np.full(GW, -1, np.int64)
    for r, g in enumerate(order):
        wpos = r // SLOT_G
        q = r % SLOT_G
        core_of_w[g] = q % NCORES
        pass_of_w[g] = q // NCORES
        wpos_of_w[g] = wpos
    pos_of_w = pass_of_w * WPP + wpos_of_w            # position in [0, WPC)

    # permutation: node -> row in the AllGathered table.
    # Table order: (pass-half, core, slot-within-half, local).
    nodes = np.arange(N, dtype=np.int64)
    gs = nodes // P
    pos_n = pos_of_w[gs]
    half_n = pos_n // (WPC // 2)
    perm = (half_n * THALF + core_of_w[gs] * HROWS
            + (pos_n - half_n * (WPC // 2)) * P + (nodes - gs * P))

    # ----- per-edge cell keys, one set per layer style -----
    e_pos = pos_of_w[gdst]
    e_core = core_of_w[gdst]
    e_pass = e_pos // WPP
    e_wpos = e_pos - e_pass * WPP
    e_dstl = (dst - gdst * P).astype(np.float32)
    r_src = perm[src]

    def build(style):
        # style "l1": chunk = r // CH (quarters), idx = r % CH
        # style "l23": chunk = (pair-half)*2+(parity), idx = pair % 25600
        if style == "l1":
            e_chunk = r_src // CH
            e_idx = (r_src - e_chunk * CH).astype(np.int16)
        else:
            pr = r_src >> 1
            par = r_src & 1
            c2 = pr // (THALF // 2)
            e_chunk = c2 * 2 + par
            e_idx = (pr - c2 * (THALF // 2)).astype(np.int16)

        cell = ((e_core * NPASS + e_pass) * NCHUNK + e_chunk) * WPP + e_wpos
        ncells = NCORES * NPASS * NCHUNK * WPP
        cnt = np.bincount(cell, minlength=ncells).reshape(
            NCORES, NPASS, NCHUNK, WPP)

        tiles = (cnt + P - 1) // P
        B = tiles.max(axis=(0, 1)).T.copy()           # [WPP, NCHUNK]
        T_c = B.sum(axis=0)                           # tiles per chunk
        T_P = int(T_c.sum())                          # tiles per pass

        cell_sizes = np.zeros((NCORES, NPASS, NCHUNK, WPP), np.int64)
        cell_sizes[:, :, :, :] = (B.T[None, None] * P)
        flat_sizes = cell_sizes.reshape(-1)
        cell_off = np.zeros(ncells, np.int64)
        cell_off[1:] = np.cumsum(flat_sizes)[:-1]

        nslots_core = NPASS * T_P * P

        sort_idx = np.argsort(cell, kind="stable")
        cell_sorted = cell[sort_idx]
        starts = np.searchsorted(cell_sorted, np.arange(ncells))
        within = np.arange(E, dtype=np.int64) - starts[cell_sorted]
        out_pos_sorted = cell_off[cell_sorted] + within
        out_pos = np.empty(E, np.int64)
        out_pos[sort_idx] = out_pos_sorted

        tot_slots = NCORES * nslots_core
        s_idx = np.zeros(tot_slots, np.int16)
        s_dstl = np.zeros(tot_slots, np.float32)
        s_w = np.zeros(tot_slots, np.float32)
        s_idx[out_pos] = e_idx
        s_dstl[out_pos] = e_dstl
        s_w[out_pos] = w1

        idx_tbls, meta1_tbls, meta2_tbls = [], [], []
        for c in range(NCORES):
            lo = c * nslots_core
            ci = s_idx[lo:lo + nslots_core].reshape(NPASS, T_P * P)
            cd = s_dstl[lo:lo + nslots_core].reshape(NPASS, T_P, P)
            cw = s_w[lo:lo + nslots_core].reshape(NPASS, T_P, P)
            blocks = []
            for i in range(NPASS):
                blocks.append(ci[i].reshape(T_P * 8, 16).T)     # [16, T_P*8]
            blocks.append(np.zeros((16, T_P * 8), np.int16))    # dummy pass
            idx2 = np.concatenate(blocks, axis=1)
            idx_tbls.append(np.tile(idx2, (8, 1)))              # [128, ...]

            def pack_meta(warr):
                m = np.zeros((NPASS, P, 2 * T_P), np.float32)
                for i in range(NPASS):
                    m[i, :, 0::2] = cd[i].T
                    m[i, :, 1::2] = warr[i].T
                return m.transpose(1, 0, 2).reshape(P, NPASS * 2 * T_P)

            meta1_tbls.append(pack_meta(cw))
            meta2_tbls.append(pack_meta(2.0 * cw))

        return dict(B=B, T_c=[int(x) for x in T_c], T_P=T_P,
                    idx=idx_tbls, meta1=meta1_tbls, meta2=meta2_tbls)

    info = dict(perm=perm, core_of_w=core_of_w, pos_of_w=pos_of_w,
                l1=build("l1"), l23=build("l23"))
    return info


def _permute_x(x: np.ndarray, info):
    """x [N, FIN] -> per-core [NR, FIN] shards in slot order."""
    xs = [np.zeros((NR, FIN), np.float32) for _ in range(NCORES)]
    core_of_w, pos_of_w = info["core_of_w"], info["pos_of_w"]
    for g in range(GW):
        c, p = core_of_w[g], pos_of_w[g]
        lo, hi = g * P, min((g + 1) * P, N)
        xs[c][p * P:p * P + (hi - lo)] = x[lo:hi]
    return xs


def _assemble_out(shards, info):
    out = np.zeros((N, OUT), np.float32)
    core_of_w, pos_of_w = info["core_of_w"], info["pos_of_w"]
    for g in range(GW):
        c, p = core_of_w[g], pos_of_w[g]
        lo, hi = g * P, min((g + 1) * P, N)
        out[lo:hi] = shards[c][p * P:p * P + (hi - lo), :OUT]
    return out


# =====================================================================
# Numpy emulation of the device data layout (validates the tables)
# =====================================================================
def _emu_prop(info, lset, tbl, w_sel, fpad, style):
    """tbl: [TROWS, fpad] table (node rows, compact); per-core [NR, fpad]."""
    T_P = lset["T_P"]
    B = lset["B"]
    outs = []
    for c in range(NCORES):
        idx = lset["idx"][c][:16]
        meta = lset[w_sel][c]
        out = np.zeros((NR, fpad), np.float32)
        for i in range(NPASS):
            ib = idx[:, i * T_P * 8:(i + 1) * T_P * 8]
            idxs = ib.T.reshape(-1).astype(np.int64)
            mb = meta[:, i * 2 * T_P:(i + 1) * 2 * T_P]
            off = 0
            for ci in range(NCHUNK):
                Tc = int(B[:, ci].sum())
                for t in range(Tc):
                    tg = off + t
                    eidx = idxs[tg * P:(tg + 1) * P]
                    acc, wp = 0, 0
                    while t >= acc + B[wp, ci]:
                        acc += B[wp, ci]
                        wp += 1
                    dstl = mb[:, 2 * tg].astype(np.int64)
                    wv = mb[:, 2 * tg + 1]
                    if style == "l1":
                        rows = ci * CH + eidx
                    else:
                        c2, par = ci // 2, ci % 2
                        rows = c2 * THALF + 2 * eidx + par
                    g = tbl[rows]
                    z = g * wv[:, None]
                    dr = (i * WPP + wp) * P + dstl
                    np.add.at(out, dr, z)
                off += Tc
        outs.append(out)
    return outs


def _emulate(x, info, W1, b1, W2, b2, W3, b3, bf16_round=False):
    """Numpy emulation with device data layout. Returns [N, OUT]."""
    def rnd(a):
        if not bf16_round:
            return a
        b = np.asarray(a, np.float32).copy()
        v = b.view(np.uint32)
        v += 0x8000
        v &= 0xFFFF0000
        return b

    mean = x.mean(axis=0)
    std = x.std(axis=0, ddof=1)
    xs = _permute_x(x, info)
    hs = [(s - mean) / std for s in xs]
    perm = info["perm"]
    core_of_w, pos_of_w = info["core_of_w"], info["pos_of_w"]

    def to_table(mats, fpad):
        tbl = np.zeros((TROWS, fpad), np.float32)
        for c in range(NCORES):
            rows = np.arange(NR)
            half = rows // HROWS
            trow = half * THALF + c * HROWS + (rows - half * HROWS)
            tbl[trow] = mats[c][:, :fpad]
        return tbl

    def layer(hs, W, bias, l, relu):
        Kk = W.shape[0]
        fout = W.shape[2]
        fpad = F_PAD[l]
        style = "l1" if l == 1 else "l23"
        lset = info["l1" if l == 1 else "l23"]

        def pad(mats):
            return [np.concatenate([m, np.zeros((NR, fpad - m.shape[1]),
                                                np.float32)], 1)
                    if m.shape[1] < fpad else m for m in mats]

        C = [[h @ W[k] for h in hs] for k in range(Kk)]
        b_kp1 = None
        b_kp2 = None
        for s in range(Kk):
            k = Kk - 1 - s
            if s == 0:
                b_k = [rnd(v) for v in pad(C[k])]
            else:
                w_sel = "meta1" if s == Kk - 1 else "meta2"
                tbl = to_table(b_kp1, fpad)
                prop = _emu_prop(info, lset, tbl, w_sel, fpad, style)
                b_k = []
                for c in range(NCORES):
                    v = prop[c]
                    v[:, :fout] += C[k][c]
                    if b_kp2 is not None:
                        v = v - b_kp2[c]
                    b_k.append(rnd(v) if s < Kk - 1 else v)
            b_kp2, b_kp1 = b_kp1, b_k
        outs = []
        for c in range(NCORES):
            v = b_kp1[c][:, :fout] + bias[None, :]
            if relu:
                v = np.maximum(v, 0.0)
            outs.append(v)
        return outs

    hs = layer(hs, W1, b1, 1, True)
    hs = layer(hs, W2, b2, 2, True)
    hs = layer(hs, W3, b3, 3, False)
    return _assemble_out(hs, info)


# =====================================================================
# Bass kernel
# =====================================================================
def _build_nc(info):
    import concourse.bass as bass
    import concourse.mybir as mybir
    import concourse.tile as tile
    from concourse import bacc
    from concourse.bass import ds

    f32 = mybir.dt.float32
    bf16 = mybir.dt.bfloat16
    DT = {"f32": f32, "bf16": bf16}

    l1set, l23set = info["l1"], info["l23"]
    T_P1, T_P2 = l1set["T_P"], l23set["T_P"]

    nc = bacc.Bacc(None, target_bir_lowering=False, num_swdge_queues=4)

    # ---- I/O ----
    xs_d = nc.dram_tensor("xs", [NR, FIN], f32, kind="ExternalInput")
    idx1_d = nc.dram_tensor("idx1", [P, (NPASS + 1) * T_P1 * 8],
                            mybir.dt.int16, kind="ExternalInput")
    idx2_d = nc.dram_tensor("idx2", [P, (NPASS + 1) * T_P2 * 8],
                            mybir.dt.int16, kind="ExternalInput")
    meta_d = {}
    for nm, tp in (("m1a", T_P1), ("m2a", T_P1), ("m1b", T_P2), ("m2b", T_P2)):
        meta_d[nm] = nc.dram_tensor(nm, [P, NPASS * 2 * tp], f32,
                                    kind="ExternalInput")
    CW = 128 + 128 + 128 + 32 + 40 + 1
    consts_d = nc.dram_tensor("consts", [P, CW], f32, kind="ExternalInput")
    w_d = {
        1: nc.dram_tensor("w1", [FIN, K * HID], f32, kind="ExternalInput"),
        2: nc.dram_tensor("w2", [HID, K * F2], f32, kind="ExternalInput"),
        3: nc.dram_tensor("w3", [F2, K * OUT], f32, kind="ExternalInput"),
    }
    out_d = nc.dram_tensor("out_shard", [NR, OUT], f32, kind="ExternalOutput")

    # ---- internal DRAM ----
    # per-half shard buffers + AllGathered table halves, by row bytes.
    widths = sorted({ROW_B[l] if PROP_DT[l] == "bf16" else 2 * ROW_B[l]
                     for l in (1, 2, 3)})
    tshh = {w: [[nc.dram_tensor(f"tsh{w}_{j}_{h}", [HROWS, w // 4], f32)
                 for h in range(2)] for j in range(2)] for w in widths}
    tbh = {w: [[nc.dram_tensor(f"tbh{w}_{j}_{h}", [THALF + 2, w // 4], f32,
                               addr_space="Shared")
                for h in range(2)] for j in range(2)] for w in widths}
    st_in = nc.dram_tensor("st_in", [P, 2], f32)
    st_out = nc.dram_tensor("st_out", [P, 2], f32, addr_space="Shared")

    RG = [[0, 1, 2, 3, 4, 5, 6, 7]]

    with tile.TileContext(nc) as tc:
        with tc.tile_pool(name="per", bufs=1) as per, \
             tc.tile_pool(name="big", bufs=1) as bigp, \
             tc.tile_pool(name="str", bufs=2) as strm, \
             tc.tile_pool(name="Sp", bufs=4) as Sp, \
             tc.tile_pool(name="gb", bufs=1) as gbp, \
             tc.tile_pool(name="ps", bufs=5, space="PSUM") as psp, \
             tc.tile_pool(name="pst", bufs=1, space="PSUM") as pst:

            # ---------- persistent constants ----------
            consts = per.tile([P, CW], f32, tag="consts")
            nc.sync.dma_start(consts[:], consts_d[:])
            iota_f32 = consts[:, 0:128]
            ident = consts[:, 128:256]
            biases = {1: consts[:, 256:384], 2: consts[:, 384:416],
                      3: consts[:, 416:456]}
            ones_col = consts[:, 456:457]

            wmat = {}
            for l in (1, 2, 3):
                fin = LAYER_FIN[l]
                wm = per.tile([P, K * LAYER_FOUT[l]], f32, tag=f"wm{l}")
                nc.sync.dma_start(wm[:fin, :], w_d[l][:])
                wmat[l] = wm

            iota_bf = per.tile([P, 128], bf16, tag="iotabf")
            nc.vector.tensor_copy(iota_bf[:], iota_f32)

            # hT master buffer [128, WPC*128] f32
            hT = bigp.tile([P, WPC * P], f32, tag="hT")

            # ---------- phase A: stats + transpose of x ----------
            ps_stat = pst.tile([P, 2], f32, space="PSUM", tag="stat")
            for p in range(WPC):
                xt = strm.tile([P, FIN], f32, tag="xt")
                nc.sync.dma_start(xt[:], xs_d[p * P:(p + 1) * P, :])
                sq = strm.tile([P, FIN], f32, tag="sq")
                nc.vector.tensor_tensor(out=sq[:], in0=xt[:], in1=xt[:],
                                        op=mybir.AluOpType.mult)
                nc.tensor.matmul(ps_stat[:, 0:1], lhsT=xt[:], rhs=ones_col,
                                 start=(p == 0), stop=False)
                nc.tensor.matmul(ps_stat[:, 1:2], lhsT=sq[:], rhs=ones_col,
                                 start=(p == 0), stop=(p == WPC - 1))
                ps_t = pst.tile([P, P], f32, space="PSUM", tag="tp", bufs=2)
                nc.tensor.transpose(out=ps_t[:], in_=xt[:], identity=ident)
                nc.vector.tensor_copy(hT[:, p * P:(p + 1) * P], ps_t[:])

            stat_sb = per.tile([P, 2], f32, tag="statsb")
            nc.vector.tensor_copy(stat_sb[:], ps_stat[:])
            nc.sync.dma_start(st_in[:], stat_sb[:])
            nc.gpsimd.collective_compute("AllReduce", mybir.AluOpType.add,
                                         replica_groups=RG,
                                         ins=[st_in[:]], outs=[st_out[:]])
            stat2 = per.tile([P, 2], f32, tag="stat2")
            nc.sync.dma_start(stat2[:], st_out[:])
            mean = per.tile([P, 1], f32, tag="mean")
            nc.vector.tensor_scalar(out=mean[:], in0=stat2[:, 0:1],
                                    scalar1=1.0 / N, scalar2=None,
                                    op0=mybir.AluOpType.mult)
            va = per.tile([P, 1], f32, tag="va")
            nc.vector.tensor_scalar(out=va[:], in0=stat2[:, 1:2],
                                    scalar1=1.0 / (N - 1), scalar2=None,
                                    op0=mybir.AluOpType.mult)
            vb = per.tile([P, 1], f32, tag="vb")
            nc.vector.tensor_tensor(out=vb[:], in0=stat2[:, 0:1],
                                    in1=stat2[:, 0:1], op=mybir.AluOpType.mult)
            nc.vector.tensor_scalar(out=vb[:], in0=vb[:],
                                    scalar1=1.0 / (float(N) * (N - 1)),
                                    scalar2=None, op0=mybir.AluOpType.mult)
            nc.vector.tensor_tensor(out=va[:], in0=va[:], in1=vb[:],
                                    op=mybir.AluOpType.subtract)
            rstd = per.tile([P, 1], f32, tag="rstd")
            nc.scalar.activation(rstd[:], va[:],
                                 mybir.ActivationFunctionType.Sqrt)
            nc.vector.reciprocal(rstd[:], rstd[:])
            nc.vector.tensor_scalar(out=hT[:], in0=hT[:], scalar1=mean[:],
                                    scalar2=rstd[:],
                                    op0=mybir.AluOpType.subtract,
                                    op1=mybir.AluOpType.mult)

            # ---------- per-layer Clenshaw ----------
            def run_layer(l, relu):
                fin = LAYER_FIN[l]
                fout = LAYER_FOUT[l]
                fpad = F_PAD[l]
                dt = DT[PROP_DT[l]]
                is_bf = (dt == bf16)
                iota_t = iota_bf if is_bf else iota_f32
                dsz = 2 if is_bf else 4
                WB = fpad * dsz                 # table/tsh row bytes
                row_e = WB // dsz               # row elements (= fpad)
                lset = l1set if l == 1 else l23set
                T_P = lset["T_P"]
                B = lset["B"]
                T_c = lset["T_c"]
                idx_d = idx1_d if l == 1 else idx2_d
                wsel_m = {"meta1": "m1a" if l == 1 else "m1b",
                          "meta2": "m2a" if l == 1 else "m2b"}
                l23_mode = (l != 1)             # pair-view gather

                def tsh_v(j, h):
                    t = tshh[WB][j][h][:]
                    if is_bf:
                        t = t.bitcast(bf16)
                    return t                    # [HROWS, row_e]

                def gather_in(src, c):
                    # in_ap for chunk c of the prev-parity table
                    if not l23_mode:
                        h, lc = c // 2, c % 2
                        t = tbh[WB][src][h][:]
                        if is_bf:
                            t = t.bitcast(bf16)
                        return t[lc * CH:(lc + 1) * CH, :GELEM]
                    c2, par = c // 2, c % 2
                    t = tbh[WB][src][c2][:]
                    if is_bf:
                        t = t.bitcast(bf16)     # [THALF+2, 64]
                    t = t[par:par + THALF]      # [THALF, 64]
                    return t.rearrange("(n two) f -> n (two f)", two=2)

                def step(s):
                    k = K - 1 - s
                    final = (s == K - 1)
                    wsel = "meta1" if final else "meta2"
                    cur = s % 2
                    src = (s - 1) % 2
                    prevp = s % 2

                    if s == 0:
                        for i in range(NPASS):
                            _pass_body_s0(i)
                            if i == NPASS // 2 - 1:
                                _ag(cur, 0)
                        _ag(cur, 1)
                        return

                    _gathers(0, src, 0)
                    with tc.For_i(0, NPASS // 2, 2) as i:
                        _gathers(i + 1, src, 1)
                        _compute(i, 0, s, k, final, wsel, cur, prevp, 0)
                        _gathers(i + 2, src, 0)
                        _compute(i + 1, 1, s, k, final, wsel, cur, prevp, 0)
                    if not final:
                        _ag(cur, 0)
                    with tc.For_i(NPASS // 2, NPASS, 2) as i:
                        _gathers(i + 1, src, 1)
                        _compute(i, 0, s, k, final, wsel, cur, prevp, 1)
                        _gathers(i + 2, src, 0)
                        _compute(i + 1, 1, s, k, final, wsel, cur, prevp, 1)
                    if not final:
                        _ag(cur, 1)

                def _ag(j, h):
                    nc.gpsimd.collective_compute(
                        "AllGather", mybir.AluOpType.bypass, replica_groups=RG,
                        ins=[tshh[WB][j][h][:]],
                        outs=[tbh[WB][j][h][0:THALF]])

                def _gathers(i, src, par):
                    idx_t = gb_idx[par]
                    nc.sync.dma_start(
                        idx_t[:], idx_d[:, ds(i * (T_P * 8), T_P * 8)])
                    off = 0
                    for c in range(NCHUNK):
                        tc_ = T_c[c]
                        if tc_ == 0:
                            continue
                        nc.gpsimd.dma_gather(
                            out_ap=gbuf[par][:, off:off + tc_, :],
                            in_ap=gather_in(src, c),
                            idxs_ap=idx_t[:, off * 8:(off + tc_) * 8],
                            num_idxs=tc_ * P,
                            num_idxs_reg=tc_ * P,
                            elem_size=GELEM,
                            single_packet=False,
                            queue_num=c,
                        )
                        off += tc_

                def _pass_body_s0(i):
                    h = i // (NPASS // 2)
                    hs = strm.tile([P, WPP * P], f32, tag="hstg")
                    nc.sync.dma_start(hs[:fin, :],
                                      hT[:fin, i * WPP * P:(i + 1) * WPP * P])
                    hsd = hs
                    if is_bf:
                        hsd = strm.tile([P, WPP * P], bf16, tag="hstgb")
                        nc.vector.tensor_copy(hsd[:fin, :], hs[:fin, :])
                    stage = strm.tile([P, WPP * fpad], f32, tag="stg")
                    for wp in range(WPP):
                        pw = psp.tile([P, fpad], f32, space="PSUM", tag="pw")
                        nc.tensor.matmul(
                            pw[:, :fout],
                            lhsT=hsd[:fin, wp * P:(wp + 1) * P],
                            rhs=wm_l[:fin, (K - 1) * fout:K * fout],
                            start=True, stop=True)
                        nc.vector.tensor_copy(stage[:, wp * fpad:(wp + 1) * fpad],
                                              pw[:])
                    stg_o = stage
                    if is_bf:
                        stg_o = strm.tile([P, WPP * fpad], bf16, tag="stgb")
                        nc.vector.tensor_copy(stg_o[:], stage[:])
                    nc.sync.dma_start(
                        tsh_v(_cur[0], h).rearrange("(w p) f -> p w f", p=P)[
                            :, (i - h * (NPASS // 2)) * WPP:
                               (i - h * (NPASS // 2) + 1) * WPP, :row_e],
                        stg_o[:].rearrange("p (w f) -> p w f", f=row_e))

                def _compute(i, par, s, k, final, wsel, cur, prevp, h):
                    # i is For_i var over [h*10, h*10+10)
                    hs = strm.tile([P, WPP * P], f32, tag="hstg")
                    nc.sync.dma_start(hs[:fin, :],
                                      hT[:fin, ds(i * (WPP * P), WPP * P)])
                    hsd = hs
                    if is_bf:
                        hsd = strm.tile([P, WPP * P], bf16, tag="hstgb")
                        nc.vector.tensor_copy(hsd[:fin, :], hs[:fin, :])
                    meta_t = strm.tile([P, 2 * T_P], f32, tag="meta")
                    nc.sync.dma_start(
                        meta_t[:],
                        meta_d[wsel_m[wsel]][:, ds(i * (2 * T_P), 2 * T_P)])
                    meta_v = meta_t
                    if s >= 2:
                        prev = strm.tile([P, WPP * fpad], f32, tag="prev")
                        pv = tsh_v(prevp, h).rearrange("(w p) f -> p w f", p=P)[
                            :, ds(i * WPP - h * (WPC // 2), WPP), :row_e]
                        if is_bf:
                            prevb = strm.tile([P, WPP * fpad], bf16, tag="prevb")
                            nc.sync.dma_start(
                                prevb[:].rearrange("p (w f) -> p w f", f=fpad),
                                pv)
                            nc.vector.tensor_copy(prev[:], prevb[:])
                        else:
                            nc.sync.dma_start(
                                prev[:].rearrange("p (w f) -> p w f", f=fpad),
                                pv)

                    last_c = [max([c for c in range(NCHUNK) if B[wp, c] > 0],
                                  default=-1) for wp in range(WPP)]
                    pws = []
                    for wp in range(WPP):
                        pw = psp.tile([P, fpad], f32, space="PSUM", tag="pw")
                        pws.append(pw)
                        nc.tensor.matmul(
                            pw[:, :fout],
                            lhsT=hsd[:fin, wp * P:(wp + 1) * P],
                            rhs=wm_l[:fin, k * fout:(k + 1) * fout],
                            start=True, stop=(last_c[wp] < 0))
                    off = 0
                    for c in range(NCHUNK):
                        for wp in range(WPP):
                            nt = int(B[wp, c])
                            for j in range(nt):
                                tg = off
                                off += 1
                                S = Sp.tile([P, P], dt, tag="S")
                                nc.vector.tensor_scalar(
                                    out=S[:], in0=iota_t,
                                    scalar1=meta_v[:, 2 * tg:2 * tg + 1],
                                    scalar2=meta_v[:, 2 * tg + 1:2 * tg + 2],
                                    op0=mybir.AluOpType.is_equal,
                                    op1=mybir.AluOpType.mult)
                                last = (c == last_c[wp]) and (j == nt - 1)
                                nc.tensor.matmul(
                                    pws[wp][:], lhsT=S[:],
                                    rhs=gbuf[par][:, tg, :fpad],
                                    start=False, stop=last)
                    stage = strm.tile([P, WPP * fpad], f32, tag="stg")
                    for wp in range(WPP):
                        dst_sl = stage[:, wp * fpad:(wp + 1) * fpad]
                        if s >= 2:
                            nc.vector.tensor_tensor(
                                out=dst_sl, in0=pws[wp][:],
                                in1=prev[:, wp * fpad:(wp + 1) * fpad],
                                op=mybir.AluOpType.subtract)
                        else:
                            nc.vector.tensor_copy(dst_sl, pws[wp][:])
                        if final:
                            nc.vector.tensor_tensor(
                                out=dst_sl[:, :fout], in0=dst_sl[:, :fout],
                                in1=biases[l][:, :fout],
                                op=mybir.AluOpType.add)
                            if relu:
                                nc.vector.tensor_scalar(
                                    out=dst_sl[:, :fout], in0=dst_sl[:, :fout],
                                    scalar1=0.0, scalar2=None,
                                    op0=mybir.AluOpType.max)
                    if not final:
                        stg_o = stage
                        if is_bf:
                            stg_o = strm.tile([P, WPP * fpad], bf16, tag="stgb")
                            nc.vector.tensor_copy(stg_o[:], stage[:])
                        nc.sync.dma_start(
                            tsh_v(cur, h).rearrange("(w p) f -> p w f", p=P)[
                                :, ds(i * WPP - h * (WPC // 2), WPP), :row_e],
                            stg_o[:].rearrange("p (w f) -> p w f", f=row_e))
                    else:
                        if l == 3:
                            nc.sync.dma_start(
                                out_d[:].rearrange("(w p) f -> p w f", p=P)[
                                    :, ds(i * WPP, WPP), :],
                                stage[:].rearrange("p (w f) -> p w f",
                                                   f=fpad)[:, :, :OUT])
                        else:
                            for wp in range(WPP):
                                ps_t = pst.tile([P, P], f32, space="PSUM",
                                                tag="tp", bufs=2)
                                nc.tensor.transpose(
                                    out=ps_t[:fout, :],
                                    in_=stage[:, wp * fpad:wp * fpad + fout],
                                    identity=ident)
                                htn = strm.tile([P, P], f32, tag="htn")
                                nc.vector.tensor_copy(htn[:fout, :],
                                                      ps_t[:fout, :])
                                nc.sync.dma_start(
                                    hT[:fout, ds((i * WPP + wp) * P, P)],
                                    htn[:fout, :])

                gbuf = []
                gb_idx = []
                for j in range(2):
                    gtile = gbp.tile([P, T_P, GELEM], dt, tag=f"gb{j}")
                    gbuf.append(gtile)
                    gitile = gbp.tile([P, T_P * 8], mybir.dt.int16,
                                      tag=f"gi{j}")
                    gb_idx.append(gitile)
                wm_l = wmat[l]
                if is_bf:
                    wmb = per.tile([P, K * fout], bf16, tag=f"wmb{l}")
                    nc.vector.tensor_copy(wmb[:fin, :], wm_l[:fin, :])
                    wm_l = wmb

                for s in range(K):
                    if TRUNC >= 0 and _steps_done[0] >= TRUNC:
                        return
                    _cur[0] = s % 2
                    step(s)
                    _steps_done[0] += 1

            _steps_done = [0]
            _cur = [0]
            run_layer(1, relu=True)
            run_layer(2, relu=True)
            run_layer(3, relu=False)

    nc.compile()
    return nc


# =====================================================================
# Entry point
# =====================================================================
def _consts_np(b1, b2, b3):
    iota = np.tile(np.arange(P, dtype=np.float32)[None, :], (P, 1))
    CW = 128 + 128 + 128 + 32 + 40 + 1
    consts = np.zeros((P, CW), np.float32)
    consts[:, 0:128] = iota
    consts[:, 128:256] = np.eye(P, dtype=np.float32)
    consts[:, 256:256 + HID] = np.tile(np.asarray(b1, np.float32)[None, :], (P, 1))
    consts[:, 384:384 + F2] = np.tile(np.asarray(b2, np.float32)[None, :], (P, 1))
    consts[:, 416:416 + OUT] = np.tile(np.asarray(b3, np.float32)[None, :], (P, 1))
    consts[:, 456] = 1.0
    return consts


def _in_maps(info, xs, consts, W1, W2, W3):
    w1m = np.ascontiguousarray(
        np.asarray(W1, np.float32).transpose(1, 0, 2).reshape(FIN, K * HID))
    w2m = np.ascontiguousarray(
        np.asarray(W2, np.float32).transpose(1, 0, 2).reshape(HID, K * F2))
    w3m = np.ascontiguousarray(
        np.asarray(W3, np.float32).transpose(1, 0, 2).reshape(F2, K * OUT))
    l1, l23 = info["l1"], info["l23"]
    maps = []
    for c in range(NCORES):
        maps.append({
            "xs": xs[c], "idx1": l1["idx"][c], "idx2": l23["idx"][c],
            "m1a": l1["meta1"][c], "m2a": l1["meta2"][c],
            "m1b": l23["meta1"][c], "m2b": l23["meta2"][c],
            "consts": consts, "w1": w1m, "w2": w2m, "w3": w3m,
        })
    return maps


def kernel(x, edge_index, W1, b1, W2, b2, W3, b3):
    from concourse.bass_utils import run_bass_kernel_spmd

    x = np.asarray(x, np.float32)
    info = _prep(np.asarray(edge_index))
    xs = _permute_x(x, info)
    consts = _consts_np(b1, b2, b3)
    nc = _build_nc(info)
    maps = _in_maps(info, xs, consts, W1, W2, W3)
    res = run_bass_kernel_spmd(nc, maps, list(range(NCORES)))
    shards = [res.results[c]["out_shard"] for c in range(NCORES)]
    return _assemble_out(shards, info)
